# revision 38
# baseline (speedup 1.0000x reference)
"""Trainium2 Bass kernel for nn_AttnGreedySearch (attn greedy top-1 search).

Math restructure (exact in exact arithmetic):
  With A_t = W_k^t and c_t = b_k @ sum_{i<t} W_k^i (row form), the iterated
  corpus is ic_t = ic0 @ A_t + c_t where ic0 = X @ W_proj + b_proj.  Define
  the per-sample query column u~_j = A_{j+1} @ S_j with S_j = user + sum v_i
  (unnormalized running sum; positive scale + constant shift never change the
  argmax; softmax is monotonic so it is argmax-irrelevant).  Per iteration j:
      score'_j[s] = <ic0[s, :], u~_j>          (argmax-equivalent scores)
      g_j = ic0[argmax]                         (one-hot select, 16-dim)
      v_j = A_{j+1}^T g_j + c_{j+1}             (output row, exact)
      u~_{j+1} = W_k u~_j + M_j g_j + d_j,      M_j = A_{j+2} A_{j+1}^T,
                                                d_j = A_{j+2} c_{j+1}

Performance structure (fp16, rel-err budget ~1.4e-2 < 2e-2, host-validated):
  - Host pre-casts X to fp16 and pre-transposes to XT [10, 101, B] with a
    ones row (bias folded into the contraction).  Halves HBM traffic and
    removes all on-device PE transposes of the corpus.
  - P1: per item-tile matmul(lhsT=XT chunk [101,128] stationary, rhs=waug
    [101,16] moving) -> psum [128,16] is ALREADY sample-major; one ACT copy
    per 128-sample tile lands ic0a fp16 in SBUF.  LDW-bound (~64cyc FWL).
  - P2 per group of 1024 samples (8 blocks of 128 on the free axis), five
    chained iterations; groups run in staggered pairs so two dependency
    chains interleave.  prod/select are fp16 2x-mode DVE tensor-tensor ops,
    the score reduce is a DVE pool / GPSIMD reduce (alternating), the one-hot
    expansion rides ACT, the 10->1 select reduce is a pairwise fp16 TT tree,
    and the 16-dim recurrence stays on PE as 128x128 block-diagonal fp16
    matmuls.
  - All DMA is batched into ~28 large 3D-AP transfers (dma_start costs
    ~625ns of SP sequencer time each).
"""

import numpy as np

import concourse.bass as bass
import concourse.mybir as mybir
import concourse.tile as tile
from concourse import bacc
from concourse.bass_utils import run_bass_kernel_spmd
from concourse.masks import make_identity

F32 = mybir.dt.float32
F16 = mybir.dt.float16
SEARCH_NUM = 5
NCORES = 8
D = 100   # item feature dim
DA = D + 1
NSI = 10  # items per sample
H = 16    # projected dim
SH = NSI * H  # 160


def _host_constants(W_proj, b_proj, W_k, b_k):
    Wk = W_k.astype(np.float64)
    bk = b_k.astype(np.float64)
    A = [np.eye(H)]
    for _ in range(SEARCH_NUM + 1):
        A.append(A[-1] @ Wk)
    c = [np.zeros(H)]
    for _ in range(SEARCH_NUM + 1):
        c.append(c[-1] @ Wk + bk)

    def blkdiag8(m):
        out = np.zeros((128, 128))
        for t in range(8):
            out[t * H:(t + 1) * H, t * H:(t + 1) * H] = m
        return out.astype(np.float16)

    # blks packed [128, 10*128] fp16: wk, a0..a4, m0..m3
    blks = [blkdiag8(Wk.T)]
    for j in range(SEARCH_NUM):
        blks.append(blkdiag8(A[j + 1]))
    for j in range(SEARCH_NUM - 1):
        Mj = A[j + 2] @ A[j + 1].T
        blks.append(blkdiag8(Mj.T))
    blks = np.concatenate(blks, axis=1)  # [128, 1280] fp16

    # cv0..cv4, dv0..dv3 packed [128, 9] fp32
    cvdv = []
    for j in range(SEARCH_NUM):
        cvdv.append(np.tile(c[j + 1], 8).astype(np.float32)[:, None])
    for j in range(SEARCH_NUM - 1):
        dj = A[j + 2] @ c[j + 1]
        cvdv.append(np.tile(dj, 8).astype(np.float32)[:, None])
    cvdv = np.concatenate(cvdv, axis=1)  # [128, 9] fp32

    waug = np.zeros((DA, H), dtype=np.float16)
    waug[:D, :] = W_proj.astype(np.float16)
    waug[D, :] = b_proj.astype(np.float16)
    return {"blks": blks, "cvdv": cvdv, "waug": waug}


def _v(t, off, dims, nparts=None):
    """View on tile/AP t: free dims `dims`, element offset `off` added.
    `nparts` overrides the partition count (step preserved)."""
    p = list(t.ap[0])
    if nparts is not None:
        p = [p[0], nparts]
    return bass.AP(tensor=t.tensor, offset=t.offset + off,
                   ap=[p] + [list(d) for d in dims])


def build_program(nc, B):
    assert B % 1024 == 0
    NT = B // 128
    NST = B // 512
    NG = B // 1024
    mult = mybir.AluOpType.mult
    add = mybir.AluOpType.add
    iseq = mybir.AluOpType.is_equal

    # XT stored per-supertile: [NST, 101, 5248]; row r=(st,d) holds the 512
    # samples x 10 items block (i-major), padded 5120->5248 so the DRAM row
    # stride (41 x 256B pages) is coprime with the 16 DMA queues.
    XTW = NSI * 512 + 128
    xt_d = nc.dram_tensor("xt", [B // 512, 128, XTW], F16,
                          kind="ExternalInput").ap()
    user_d = nc.dram_tensor("user", [B, H], F32, kind="ExternalInput").ap()
    waug_d = nc.dram_tensor("waug", [DA, H], F16, kind="ExternalInput").ap()
    blks_d = nc.dram_tensor("blks", [128, 1280], F16, kind="ExternalInput").ap()
    cvdv_d = nc.dram_tensor("cvdv", [128, 9], F32, kind="ExternalInput").ap()
    out_d = nc.dram_tensor("out", [B, SEARCH_NUM + 1, H], F32,
                           kind="ExternalOutput").ap()

    with tile.TileContext(nc) as tc:
        with tc.tile_pool(name="singles", bufs=1) as singles, \
             tc.tile_pool(name="xst", bufs=3) as xst, \
             tc.tile_pool(name="scr", bufs=3) as scr, \
             tc.tile_pool(name="vop", bufs=2) as vop, \
             tc.tile_pool(name="ppr", bufs=4, space="PSUM") as ppr, \
             tc.tile_pool(name="pp2", bufs=2, space="PSUM") as pp2:

            # ---- persistent SBUF ----
            ident16 = singles.tile([128, 128], F16)
            make_identity(nc, ident16)
            ident32 = singles.tile([128, 128], F32)
            make_identity(nc, ident32)
            blks_sb = singles.tile([128, 1280], F16)
            nc.sync.dma_start(out=blks_sb, in_=blks_d)
            cvdv_sb = singles.tile([128, 9], F32)
            nc.sync.dma_start(out=cvdv_sb, in_=cvdv_d)
            waug_sb = singles.tile([DA, H], F16)
            nc.sync.dma_start(out=waug_sb, in_=waug_d)

            def blk_wk():
                return blks_sb[:, 0:128]

            def blk_a(j):
                return blks_sb[:, (1 + j) * 128:(2 + j) * 128]

            def blk_m(j):
                return blks_sb[:, (6 + j) * 128:(7 + j) * 128]

            def cv(j):
                return cvdv_sb[:, j:j + 1]

            def dv(j):
                return cvdv_sb[:, 5 + j:6 + j]

            ic0a = singles.tile([128, NT * SH], F16)   # (t,i,h) per tile
            usera = singles.tile([128, NG * 128], F32)  # sample-major (t,h)
            ua = singles.tile([128, NG * 128], F16)     # u~ sample-major
            ud = singles.tile([128, NG * 128], F16)     # u~ feature-major

            # one big user DMA: usera[p, (g,t,h)] = user[g*1024+t*128+p, h]
            src_ap = bass.AP(tensor=user_d.tensor, offset=user_d.offset,
                             ap=[[H, 128], [128 * H, NG * 8], [1, H]])
            nc.sync.dma_start(out=usera, in_=src_ap)

            # ---- P0: u~_0 = W_k @ user, pair-batched ----
            def emit_p0(q):
                p0 = q * 256
                tp = pp2.tile([128, 256], F32, name="tp0", tag="p2f32")
                nc.tensor.transpose(tp[:, 0:128], usera[:, p0:p0 + 128],
                                    ident32)
                nc.tensor.transpose(tp[:, 128:256], usera[:, p0 + 128:p0 + 256],
                                    ident32)
                userd16 = scr.tile([128, 256], F16, name="userd16", tag="gd")
                nc.scalar.copy(userd16, tp)
                up = pp2.tile([128, 256], F32, name="up0", tag="p2f32")
                nc.tensor.matmul(up, blk_wk(), userd16, start=True, stop=True)
                nc.scalar.copy(ud[:, p0:p0 + 256], up)
                tp2 = pp2.tile([128, 256], F16, name="tp0b", tag="p2f16")
                nc.tensor.transpose(tp2[:, 0:128], ud[:, p0:p0 + 128], ident16)
                nc.tensor.transpose(tp2[:, 128:256], ud[:, p0 + 128:p0 + 256],
                                    ident16)
                nc.scalar.copy(ua[:, p0:p0 + 256], tp2)

            # ---- P1 for one super-tile of 512 samples ----
            def emit_p1(st):
                xt_sb = xst.tile([128, NSI * 512], F16, name="xt_sb")
                src = bass.AP(tensor=xt_d.tensor,
                              offset=xt_d.offset + st * 128 * XTW,
                              ap=[[XTW, 128], [1, NSI * 512]])
                nc.sync.dma_start(out=xt_sb, in_=src)
                for a in range(4):
                    c_ = st * 4 + a
                    pc = ppr.tile([128, SH], F32, name="pc", tag="pc")
                    for i in range(NSI):
                        nc.tensor.matmul(
                            pc[:, i * H:(i + 1) * H],
                            xt_sb[:DA, i * 512 + a * 128:i * 512 + (a + 1) * 128],
                            waug_sb, start=True, stop=True)
                    nc.scalar.copy(ic0a[:, c_ * SH:(c_ + 1) * SH], pc)

            # ---- P2: one iteration for one PAIR of groups (2q, 2q+1) ----
            def emit_pair_iter(q, j):
                base = q * 16 * SH          # ic0a element offset of the pair
                p0 = q * 256                # ua/ud column offset
                ic_p = _v(ic0a, base, [[SH, 16], [H, NSI], [1, H]])
                ua_p = _v(ua, p0, [[H, 16], [0, NSI], [1, H]])

                prod = scr.tile([128, 16, NSI, H], F16, name="prod",
                                tag="prod")
                nc.vector.tensor_tensor(
                    out=_v(prod, 0, [[SH, 6], [H, NSI], [1, H]]),
                    in0=_v(ic0a, base, [[SH, 6], [H, NSI], [1, H]]),
                    in1=_v(ua, p0, [[H, 6], [0, NSI], [1, H]]), op=mult)
                nc.vector.tensor_tensor(
                    out=_v(prod, 6 * SH, [[SH, 10], [H, NSI], [1, H]]),
                    in0=_v(ic0a, base + 6 * SH, [[SH, 10], [H, NSI], [1, H]]),
                    in1=_v(ua, p0 + 6 * H, [[H, 10], [0, NSI], [1, H]]),
                    op=mult)
                scores = scr.tile([128, 16, NSI], F32, name="scores",
                                  tag="scores")
                nc.vector.reduce_sum(
                    out=_v(scores, 0, [[NSI, 6], [1, NSI]]),
                    in_=_v(prod, 0, [[SH, 6], [H, NSI], [1, H]]),
                    axis=mybir.AxisListType.X)
                mx = scr.tile([128, 16], F32, name="mx", tag="mx")
                nc.vector.reduce_max(out=_v(mx, 0, [[1, 6]]),
                                     in_=_v(scores, 0, [[NSI, 6], [1, NSI]]),
                                     axis=mybir.AxisListType.X)
                mask = scr.tile([128, 16, NSI], F32, name="mask", tag="mask")
                nc.vector.tensor_tensor(
                    out=_v(mask, 0, [[NSI, 6], [1, NSI]]),
                    in0=_v(scores, 0, [[NSI, 6], [1, NSI]]),
                    in1=_v(mx, 0, [[1, 6], [0, NSI]]), op=iseq)
                nc.vector.reduce_sum(
                    out=_v(scores, 6 * NSI, [[NSI, 10], [1, NSI]]),
                    in_=_v(prod, 6 * SH, [[SH, 10], [H, NSI], [1, H]]),
                    axis=mybir.AxisListType.X)
                nc.vector.reduce_max(out=_v(mx, 6, [[1, 10]]),
                                     in_=_v(scores, 6 * NSI,
                                            [[NSI, 10], [1, NSI]]),
                                     axis=mybir.AxisListType.X)
                nc.vector.tensor_tensor(
                    out=_v(mask, 6 * NSI, [[NSI, 10], [1, NSI]]),
                    in0=_v(scores, 6 * NSI, [[NSI, 10], [1, NSI]]),
                    in1=_v(mx, 6, [[1, 10], [0, NSI]]), op=iseq)
                mask16 = scr.tile([128, 16, NSI, H], F16, name="mask16",
                                  tag="mask16")
                # GPS half of the select is slow: expand its mask first
                ha = [[SH, 10], [H, NSI], [1, H]]
                hb = [[SH, 6], [H, NSI], [1, H]]
                nc.scalar.copy(_v(mask16, 0, ha),
                               _v(mask, 0, [[NSI, 10], [1, NSI], [0, H]]))
                nc.scalar.copy(_v(mask16, 10 * SH, hb),
                               _v(mask, 10 * NSI, [[NSI, 6], [1, NSI], [0, H]]))
                sel = scr.tile([128, 16, NSI, H], F16, name="sel", tag="sel")
                nc.gpsimd.tensor_tensor(
                    out=_v(sel, 0, ha), in0=_v(ic0a, base, ha),
                    in1=_v(mask16, 0, ha), op=mult)
                nc.vector.tensor_tensor(
                    out=_v(sel, 10 * SH, hb),
                    in0=_v(ic0a, base + 10 * SH, hb),
                    in1=_v(mask16, 10 * SH, hb), op=mult)
                # pairwise fold over items: 10 -> 5 -> (4->2->1) + leftover
                f1 = scr.tile([128, 16, 5, H], F16, name="f1", tag="f1")
                nc.vector.tensor_tensor(
                    out=f1, in0=_v(sel, 0, [[SH, 16], [H, 5], [1, H]]),
                    in1=_v(sel, 5 * H, [[SH, 16], [H, 5], [1, H]]), op=add)
                f2 = scr.tile([128, 16, 2, H], F16, name="f2", tag="f2")
                nc.vector.tensor_tensor(
                    out=f2, in0=_v(f1, 0, [[5 * H, 16], [H, 2], [1, H]]),
                    in1=_v(f1, 2 * H, [[5 * H, 16], [H, 2], [1, H]]), op=add)
                f3 = scr.tile([128, 16, H], F16, name="f3", tag="f3")
                nc.vector.tensor_tensor(
                    out=f3, in0=_v(f2, 0, [[2 * H, 16], [1, H]]),
                    in1=_v(f2, H, [[2 * H, 16], [1, H]]), op=add)
                ga = scr.tile([128, 16, H], F16, name="ga", tag="ga")
                nc.vector.tensor_tensor(
                    out=ga, in0=f3, in1=_v(f1, 4 * H, [[5 * H, 16], [1, H]]),
                    op=add)

                tpg = pp2.tile([128, 256], F16, name="tpg", tag="p2f16")
                nc.tensor.transpose(tpg[:, 0:128], _v(ga, 0, [[1, 128]]),
                                    ident16)
                nc.tensor.transpose(tpg[:, 128:256], _v(ga, 128, [[1, 128]]),
                                    ident16)
                gd16 = scr.tile([128, 256], F16, name="gd16", tag="gd")
                nc.scalar.copy(gd16, tpg)
                # u~ recurrence first: cross-iteration critical path
                if j < SEARCH_NUM - 1:
                    up = pp2.tile([128, 256], F32, name="upj", tag="p2f32")
                    nc.tensor.matmul(up[:, 0:128], blk_wk(),
                                     ud[:, p0:p0 + 128], start=True,
                                     stop=False)
                    nc.tensor.matmul(up[:, 0:128], blk_m(j), gd16[:, 0:128],
                                     start=False, stop=True)
                    nc.tensor.matmul(up[:, 128:256], blk_wk(),
                                     ud[:, p0 + 128:p0 + 256], start=True,
                                     stop=False)
                    nc.tensor.matmul(up[:, 128:256], blk_m(j),
                                     gd16[:, 128:256], start=False, stop=True)
                    nc.scalar.add(ud[:, p0:p0 + 256], up, dv(j))
                    tpu = pp2.tile([128, 256], F16, name="tpu", tag="p2f16")
                    nc.tensor.transpose(tpu[:, 0:128], ud[:, p0:p0 + 128],
                                        ident16)
                    nc.tensor.transpose(tpu[:, 128:256],
                                        ud[:, p0 + 128:p0 + 256], ident16)
                    nc.scalar.copy(ua[:, p0:p0 + 256], tpu)
                vp = pp2.tile([128, 256], F32, name="vp", tag="p2f32")
                nc.tensor.matmul(vp[:, 0:128], blk_a(j), gd16[:, 0:128],
                                 start=True, stop=True)
                nc.tensor.matmul(vp[:, 128:256], blk_a(j), gd16[:, 128:256],
                                 start=True, stop=True)
                vtmp = scr.tile([128, 256], F16, name="vtmp", tag="vtmp")
                nc.scalar.add(vtmp, vp, cv(j))
                tpv = pp2.tile([128, 256], F16, name="tpv", tag="p2f16")
                nc.tensor.transpose(tpv[:, 0:128], vtmp[:, 0:128], ident16)
                nc.tensor.transpose(tpv[:, 128:256], vtmp[:, 128:256],
                                    ident16)
                vout = vouts[q]
                nc.scalar.copy(
                    _v(vout, (1 + j) * H, [[768, 2], [96, 8], [1, H]]), tpv)
                if j == 0:
                    nc.scalar.copy(
                        _v(vout, 0, [[768, 2], [96, 8], [1, H]]),
                        _v(usera, p0, [[128, 2], [H, 8], [1, H]]))

            def emit_p3(g):
                vout = vouts[g // 2]
                dst = bass.AP(
                    tensor=out_d.tensor,
                    offset=out_d.offset + g * 1024 * 96,
                    ap=[[96, 128], [128 * 96, 8], [1, 96]])
                nc.sync.dma_start(out=dst,
                                  in_=_v(vout, (g % 2) * 768,
                                         [[96, 8], [1, 96]]))

            vouts = [vop.tile([128, 2 * 8 * 96], F32, name=f"vout{k}")
                     for k in range(4)]

            # ---- main schedule: software-pipelined pair-chains ----
            # rounds: pairs 0,1 start immediately (their supertiles are
            # loaded first); pairs 2,3 start two iterations later, once the
            # prefetched second half of P1 lands.
            # P1 for a pair's supertiles must be EMITTED before the pair's
            # first iteration (tile deps follow emission order).
            emit_p1(0)
            emit_p1(1)
            emit_p0(0)
            emit_p1(2)
            emit_p1(3)
            emit_p0(1)
            for st in range(4, min(8, NST)):
                emit_p1(st)
            emit_p0(2)
            emit_p0(3)
            next_st = 8
            rounds = [
                [(0, 0), (1, 0)],
                [(0, 1), (1, 1)],
                [(0, 2), (1, 2), (2, 0), (3, 0)],
                [(0, 3), (1, 3), (2, 1), (3, 1)],
                [(0, 4), (1, 4), (2, 2), (3, 2)],
                [(2, 3), (3, 3)],
                [(2, 4), (3, 4)],
            ]
            for r, work in enumerate(rounds):
                for q, j in work:
                    emit_pair_iter(q, j)
                    for _ in range(2):
                        if next_st < NST:
                            emit_p1(next_st)
                            next_st += 1
                    if j == SEARCH_NUM - 1:
                        emit_p3(2 * q)
                        emit_p3(2 * q + 1)


def _in_maps(inputs, B_core):
    cst = _host_constants(inputs["W_proj"], inputs["b_proj"],
                          inputs["W_k"], inputs["b_k"])
    x = np.asarray(inputs["item_corpus"], dtype=np.float32)
    u = np.ascontiguousarray(inputs["user_intent"], dtype=np.float32)
    B = x.shape[0]
    NST = B_core // 512
    XTW = NSI * 512 + 128
    # [nst_total, 100, 10, 512]: block (st, d, i, s)
    xb = (x.astype(np.float16)
          .reshape(B // 512, 512, NSI, D)
          .transpose(0, 3, 2, 1))
    maps = []
    for core in range(NCORES):
        xt = np.zeros((NST, 128, XTW), dtype=np.float16)
        xt[:, :D, :NSI * 512] = xb[core * NST:(core + 1) * NST].reshape(
            NST, D, NSI * 512)
        xt[:, D, :NSI * 512] = 1.0
        lo, hi = core * B_core, (core + 1) * B_core
        m = {"xt": xt, "user": u[lo:hi],
             "waug": cst["waug"], "blks": cst["blks"], "cvdv": cst["cvdv"]}
        maps.append(m)
    return maps


_COMPILED = {}


def _get_nc(B_core):
    if B_core not in _COMPILED:
        nc = bacc.Bacc("TRN2", target_bir_lowering=False, debug=False,
                       num_devices=NCORES)
        build_program(nc, B_core)
        nc.compile()
        _COMPILED[B_core] = nc
    return _COMPILED[B_core]


def kernel(**inputs) -> np.ndarray:
    bs = inputs["user_intent"].shape[0]
    assert bs % NCORES == 0
    B_core = bs // NCORES
    nc = _get_nc(B_core)
    res = run_bass_kernel_spmd(nc, _in_maps(inputs, B_core),
                               core_ids=list(range(NCORES)))
    out = np.concatenate([r["out"] for r in res.results], axis=0)
    return out.astype(np.float32)
